# revision 3
# baseline (speedup 1.0000x reference)
"""Self-contained Trainium2 kernel for nn_ActorCriticNetwork (2-layer LSTM).

kernel(**inputs) takes the full unsharded inputs and returns (v, mu, cov)
matching reference.reference(). The two LSTMs are pair-sharded across
NeuronCores (hidden-dim split within pairs), run fully on device in bf16 via
a Bass kernel; the tiny dense head runs on host in fp32.
"""
import contextlib
import numpy as np
from ml_dtypes import bfloat16
from concourse import bass, library_config
import concourse.bacc as bacc
import concourse.mybir as mybir
from concourse.bass_utils import run_bass_kernel_spmd

F = 64
H1, H2 = 1024, 512
B = 32
T_FULL = 1024
U1, U2 = 512, 256
M1, M2 = 16, 8
K1, K2 = 8, 4
LAG, WIN, RING = 17, 16, 32

FP32 = mybir.dt.float32
BF16 = mybir.dt.bfloat16
POS2REF = [0, 2, 1, 3]


class _Sem:
    def __init__(self, h):
        self.h, self.n = h, 0
    def inc(self, instr, amt=1):
        instr.then_inc(self.h, amt)
        self.n += amt
        return self.n


def _build_kernel(T):
    SIG = mybir.ActivationFunctionType.Sigmoid
    assert T % WIN == 0 and T >= 2 * WIN
    G_TOT = T + LAG
    G_SEND = T + LAG - 1
    nc = bacc.Bacc(detect_race_conditions=False)

    w1r_d = nc.dram_tensor("w1r", [128, M1 * K1 * 128], BF16, kind="ExternalInput")
    w1k_d = nc.dram_tensor("w1k", [65, M1 * 128], BF16, kind="ExternalInput")
    w2k_d = nc.dram_tensor("w2k", [128, M2 * K1 * 128], BF16, kind="ExternalInput")
    b2r_d = nc.dram_tensor("b2r", [1, M2 * 128], BF16, kind="ExternalInput")
    w2r_d = nc.dram_tensor("w2r", [128, M2 * K2 * 128], BF16, kind="ExternalInput")
    st_d = nc.dram_tensor("stateT", [65, T * B], BF16, kind="ExternalInput")
    out_d = nc.dram_tensor("out", [128, 64], FP32, kind="ExternalOutput")

    ctx = contextlib.ExitStack()
    sb = lambda n, s, t: ctx.enter_context(nc.sbuf_tensor(n, s, t))
    psf = lambda n, s: ctx.enter_context(nc.psum_tensor(n, s, FP32))
    mks = lambda n: _Sem(ctx.enter_context(nc.semaphore(n)))

    w1r = sb("w1r_s", [128, M1 * K1 * 128], BF16)
    w1k = sb("w1k_s", [65, M1 * 128], BF16)
    w2k = sb("w2k_s", [128, M2 * K1 * 128], BF16)
    b2r = sb("b2r_s", [1, M2 * 128], BF16)
    w2r = sb("w2r_s", [128, M2 * K2 * 128], BF16)
    st = sb("st_s", [65, T * B], BF16)
    ones = sb("ones_s", [1, WIN * B], BF16)
    ring = sb("ring_s", [128, RING * 384], BF16)
    xz2b = sb("xz2b_s", [128, 2 * M2 * WIN * B], BF16)
    sig1 = sb("sig1_s", [128, 2 * 512], FP32)
    sc1 = sb("sc1_s", [128, 2 * 128], FP32)
    c1 = sb("c1_s", [128, 128], FP32)
    tt1 = sb("tt1_s", [128, 128], FP32)
    uu1 = sb("uu1_s", [128, 128], FP32)
    z2s = sb("z2s_s", [128, 2 * 256], FP32)
    sig2 = sb("sig2_s", [128, 2 * 256], FP32)
    sc2 = sb("sc2_s", [128, 2 * 64], FP32)
    c2 = sb("c2_s", [128, 64], FP32)
    tt2 = sb("tt2_s", [128, 64], FP32)
    uu2 = sb("uu2_s", [128, 64], FP32)
    h2f = sb("h2f_s", [128, 64], FP32)

    z1p = [psf("z1a", [128, 512]), psf("z1b", [128, 512])]
    z2p = [psf("z2a", [128, 512]), psf("z2b", [128, 512])]
    xzp = [psf("xza", [128, 512]), psf("xzb", [128, 512])]

    d = mks("d"); prep = mks("prep"); lsem = mks("lsem"); rsem = mks("rsem")
    mm1 = mks("mm1"); mm2 = mks("mm2"); mmx = mks("mmx")
    aS = mks("aS"); vS = mks("vS"); dout = mks("dout")

    def rc(g, region):
        off = {"h1m": 0, "h2m": 128, "h1p": 192, "h2p": 320}[region]
        return (g % RING) * 384 + off

    def h1chunk(g, k):
        col = rc(g, "h1m") + k * 32 if k < 4 else rc(g, "h1p") + (k - 4) * 32
        return ring[:, col:col + 32]

    def h2chunk(g, k):
        col = rc(g, "h2m") + k * 32 if k < 2 else rc(g, "h2p") + (k - 2) * 32
        return ring[:, col:col + 32]

    def windows(g):
        return (g > 0 and g % WIN == 0 and g <= T)

    mm1_at = {}; mm2_at = {}; mmx_at = {}
    n = 0
    for g in range(G_TOT):
        if windows(g):
            w = g // WIN - 1
            for M in range(M2):
                n += 1; mmx_at[(w, M)] = n
    n = 0
    for g in range(T):
        for pos in range(4):
            n += 1; mm1_at[(g, pos)] = n
    n = 0
    for g in range(G_TOT):
        s = g - LAG
        if s >= 1:
            n += 1; mm2_at[s] = n

    aS_n = 0
    aS_copy = {}; aS_sg1 = {}; aS_sc1 = {}; aS_sig2 = {}; aS_sc2 = {}
    for g in range(G_TOT):
        if windows(g):
            w = g // WIN - 1
            for M in range(M2):
                aS_n += 1; aS_copy[(w, M)] = aS_n
        if g < T:
            aS_n += 1; aS_sg1[g] = aS_n
            aS_n += 1; aS_sc1[g] = aS_n
        s = g - LAG
        if s >= 0:
            aS_n += 1; aS_sig2[s] = aS_n
            aS_n += 1; aS_sc2[s] = aS_n

    vS_n = 4
    vS_uu1 = {}; vS_c1 = {}; vS_h1 = {}
    vS_z2s = {}; vS_uu2 = {}; vS_c2 = {}; vS_h2 = {}
    vS_send = {}; vS_final = None
    for g in range(G_TOT):
        if g < T:
            vS_n += 1
            vS_n += 1; vS_uu1[g] = vS_n
            vS_n += 1; vS_c1[g] = vS_n
            vS_n += 1; vS_h1[g] = vS_n
        s = g - LAG
        if s >= 0:
            vS_n += 1; vS_z2s[s] = vS_n
            vS_n += 1
            vS_n += 1; vS_uu2[s] = vS_n
            vS_n += 1; vS_c2[s] = vS_n
            vS_n += 1; vS_h2[s] = vS_n
            if s == T - 1:
                vS_n += 1; vS_final = vS_n
        vS_send[g] = vS_n

    with nc.Block() as block:
        @block.sync
        def _(sy):
            for src, dst in ((w1r_d, w1r), (w1k_d, w1k), (w2k_d, w2k),
                             (b2r_d, b2r), (w2r_d, w2r), (st_d, st)):
                d.inc(sy.dma_start(out=dst[:, :], in_=src[:, :]), 16)
            sy.wait_ge(vS.h, vS_final)
            dout.inc(sy.dma_start(out=out_d[:, :], in_=h2f[:, :]), 16)
            sy.wait_ge(dout.h, 16)
            sy.wait_ge(rsem.h, 2 * G_SEND + 2)

        @block.gpsimd
        def _(gp):
            gp.load_library(library_config.remote_dma)
            gp.wait_ge(vS.h, 4)
            rdests = [None] * 8
            rdests[1] = (0, 1)
            prep.inc(gp.remote_sem_update_broadcast(
                remote_sem=rsem.h, local_sem=lsem.h, rdests=rdests), 1)
            gp.wait_ge(prep.h, 1)
            gp.trigger_dma(1)
            gp.wait_ge(rsem.h, 2)
            def mkprep(g):
                prep.inc(gp.remote_dma_broadcast(
                    out_ap=ring[:, rc(g, "h1p"):rc(g, "h1p") + 192],
                    in_ap=ring[:, rc(g, "h1m"):rc(g, "h1m") + 192],
                    remote_sem=rsem.h, local_sem=lsem.h, rdests=rdests), 1)
            mkprep(0)
            for g in range(G_SEND):
                if g + 1 < G_SEND:
                    mkprep(g + 1)
                gp.wait_ge(prep.h, g + 2)
                gp.wait_ge(vS.h, vS_send[g])
                gp.trigger_dma(1)
            gp.wait_ge(lsem.h, 16 * (G_SEND + 1))

        @block.tensor
        def _(te):
            te.wait_ge(d.h, 6 * 16)
            te.wait_ge(vS.h, 4)
            for g in range(G_TOT):
                if windows(g):
                    w = g // WIN - 1
                    base_e = (w * WIN) % RING
                    te.wait_ge(vS.h, vS_h1[g - 1])
                    te.wait_ge(rsem.h, 2 * g + 2)
                    for M in range(M2):
                        bank = xzp[M % 2]
                        pw, pM = (w, M - 2) if M >= 2 else (w - 1, M2 - 2 + M)
                        if (pw, pM) in aS_copy:
                            te.wait_ge(aS.h, aS_copy[(pw, pM)])
                        for k in range(K1):
                            colw = (M * K1 + k) * 128
                            ch = base_e * 384 + (k * 32 if k < 4 else 192 + (k - 4) * 32)
                            rhs = bass.AP(ring, ch,
                                          [[RING * 384, 128], [384, WIN], [1, 32]])
                            te.matmul(bank[:, :], w2k[:, colw:colw + 128], rhs,
                                      start=(k == 0), stop=False)
                        mmx.inc(te.matmul(bank[:, :], b2r[:, M * 128:(M + 1) * 128],
                                          ones[:, :], start=False, stop=True), 1)
                if g < T:
                    if g >= 2:
                        te.wait_ge(aS.h, aS_sg1[g - 2])
                    if g >= 1:
                        te.wait_ge(vS.h, vS_h1[g - 1])
                        te.wait_ge(rsem.h, 2 * g + 2)
                    zb = z1p[g % 2]
                    for pos in range(4):
                        last = None
                        for ub in range(4):
                            M = pos * 4 + ub
                            oc = M * 32
                            last = te.matmul(zb[:, oc:oc + 32],
                                             w1k[:, M * 128:(M + 1) * 128],
                                             st[:, g * B:(g + 1) * B],
                                             start=True, stop=(g == 0))
                            if g > 0:
                                for k in range(K1):
                                    colw = (M * K1 + k) * 128
                                    last = te.matmul(zb[:, oc:oc + 32],
                                                     w1r[:, colw:colw + 128],
                                                     h1chunk(g - 1, k),
                                                     start=False, stop=(k == K1 - 1))
                        mm1.inc(last, 1)
                s = g - LAG
                if s >= 1:
                    if s >= 2:
                        te.wait_ge(vS.h, vS_z2s[s - 2])
                    te.wait_ge(vS.h, vS_h2[s - 1])
                    if g >= T:
                        te.wait_ge(rsem.h, (2 * g if g < G_SEND else 2 * G_SEND) + 2)
                    zb2 = z2p[s % 2]
                    last = None
                    for M in range(M2):
                        oc = M * 32
                        for k in range(K2):
                            colw = (M * K2 + k) * 128
                            last = te.matmul(zb2[:, oc:oc + 32],
                                             w2r[:, colw:colw + 128],
                                             h2chunk(g - 1, k),
                                             start=(k == 0), stop=(k == K2 - 1))
                    mm2.inc(last, 1)

        @block.scalar
        def _(sc):
            SIGf = SIG
            for g in range(G_TOT):
                if windows(g):
                    w = g // WIN - 1
                    for M in range(M2):
                        sc.wait_ge(mmx.h, mmx_at[(w, M)])
                        dst = (w % 2) * M2 * WIN * B + M * WIN * B
                        if w >= 2:
                            sc.wait_ge(vS.h, vS_z2s[(w - 2) * WIN + WIN - 1])
                        aS.inc(sc.copy(xz2b[:, dst:dst + WIN * B],
                                       xzp[M % 2][:, :]), 1)
                if g < T:
                    p = g % 2
                    if g >= 2:
                        sc.wait_ge(vS.h, vS_h1[g - 2])
                    sc.wait_ge(mm1.h, mm1_at[(g, 3)])
                    aS.inc(sc.activation(sig1[:, p * 512:(p + 1) * 512],
                                         z1p[p][:, :], SIGf), 1)
                    sc.wait_ge(vS.h, vS_c1[g])
                    aS.inc(sc.activation(sc1[:, p * 128:(p + 1) * 128],
                                         c1[:, :], SIGf), 1)
                s = g - LAG
                if s >= 0:
                    p2 = s % 2
                    if s >= 2:
                        sc.wait_ge(vS.h, vS_h2[s - 2])
                    sc.wait_ge(vS.h, vS_z2s[s])
                    aS.inc(sc.activation(sig2[:, p2 * 256:(p2 + 1) * 256],
                                         z2s[:, p2 * 256:(p2 + 1) * 256], SIGf), 1)
                    sc.wait_ge(vS.h, vS_c2[s])
                    aS.inc(sc.activation(sc2[:, p2 * 64:(p2 + 1) * 64],
                                         c2[:, :], SIGf), 1)

        @block.vector
        def _(v):
            vS.inc(v.memset(c1[:, :], 0.0), 1)
            vS.inc(v.memset(c2[:, :], 0.0), 1)
            vS.inc(v.memset(ones[:, :], 1.0), 1)
            vS.inc(v.memset(ring[:, :], 0.0), 1)
            for g in range(G_TOT):
                if g < T:
                    p = g % 2
                    sl = lambda pos: sig1[:, p * 512 + pos * 128:p * 512 + (pos + 1) * 128]
                    v.wait_ge(aS.h, aS_sg1[g])
                    if g >= 1:
                        v.wait_ge(vS.h, vS_c1[g - 1])
                    vS.inc(v.tensor_mul(tt1[:, :], sl(0), sl(1)), 1)
                    vS.inc(v.tensor_mul(uu1[:, :], sl(2), c1[:, :]), 1)
                    if g >= 1:
                        v.wait_ge(aS.h, aS_sc1[g - 1])
                    v.wait_ge(vS.h, vS_uu1[g])
                    vS.inc(v.tensor_add(c1[:, :], tt1[:, :], uu1[:, :]), 1)
                    v.wait_ge(aS.h, aS_sc1[g])
                    if g >= RING:
                        v.wait_ge(lsem.h, 16 * (g - RING + 2))
                        wdone = (g - RING) // WIN
                        v.wait_ge(mmx.h, mmx_at[(wdone, M2 - 1)])
                    vS.inc(v.tensor_mul(
                        ring[:, rc(g, "h1m"):rc(g, "h1m") + 128],
                        sl(3), sc1[:, p * 128:(p + 1) * 128]), 1)
                s = g - LAG
                if s >= 0:
                    p2 = s % 2
                    ws = (s // WIN) % 2
                    xoff = ws * M2 * WIN * B + (s % WIN) * B
                    xslice = bass.AP(xz2b, xoff, [[2 * M2 * WIN * B, 128],
                                                  [WIN * B, M2], [1, B]])
                    v.wait_ge(aS.h, aS_copy[(s // WIN, M2 - 1)])
                    if s >= 2:
                        v.wait_ge(aS.h, aS_sig2[s - 2])
                    if s == 0:
                        vS.inc(v.tensor_copy(z2s[:, 0:256], xslice), 1)
                    else:
                        v.wait_ge(mm2.h, mm2_at[s])
                        vS.inc(v.tensor_add(z2s[:, p2 * 256:(p2 + 1) * 256],
                                            z2p[p2][:, 0:256], xslice), 1)
                    s2 = lambda pos: sig2[:, p2 * 256 + pos * 64:p2 * 256 + (pos + 1) * 64]
                    v.wait_ge(aS.h, aS_sig2[s])
                    if s >= 1:
                        v.wait_ge(vS.h, vS_c2[s - 1])
                    vS.inc(v.tensor_mul(tt2[:, :], s2(0), s2(1)), 1)
                    vS.inc(v.tensor_mul(uu2[:, :], s2(2), c2[:, :]), 1)
                    v.wait_ge(vS.h, vS_uu2[s])
                    vS.inc(v.tensor_add(c2[:, :], tt2[:, :], uu2[:, :]), 1)
                    v.wait_ge(aS.h, aS_sc2[s])
                    if g >= RING:
                        v.wait_ge(lsem.h, 16 * (g - RING + 2))
                    vS.inc(v.tensor_mul(
                        ring[:, rc(g, "h2m"):rc(g, "h2m") + 64],
                        s2(3), sc2[:, p2 * 64:(p2 + 1) * 64]), 1)
                    if s == T - 1:
                        v.wait_ge(vS.h, vS_h2[s])
                        vS.inc(v.tensor_copy(
                            h2f[:, :], ring[:, rc(g, "h2m"):rc(g, "h2m") + 64]), 1)

    ctx.close()
    nc.compile()
    return nc


def _shard_inputs(T, state, W1k, W1r, b1, W2k, W2r, b2):
    bf = lambda a: np.asarray(a, dtype=bfloat16)

    def gcols(H, rho, U):
        cols = []
        for pos in range(4):
            rg = POS2REF[pos]
            lo = rg * H + rho * U
            cols.extend(range(lo, lo + U))
        return np.asarray(cols)

    stateT = np.zeros((65, T * B), np.float32)
    stateT[:64] = state[:, :T].transpose(2, 1, 0).reshape(64, T * B)
    stateT[64] = 1.0

    maps = []
    for r in range(8):
        rho = r & 1
        c1cols = gcols(H1, rho, U1)
        c2cols = gcols(H2, rho, U2)
        h1_chunks = [4 * rho + i for i in range(4)] + [4 * (1 - rho) + i for i in range(4)]
        h2_chunks = [2 * rho + i for i in range(2)] + [2 * (1 - rho) + i for i in range(2)]
        w1r_l = np.zeros((128, M1 * K1 * 128), np.float32)
        w1k_l = np.zeros((65, M1 * 128), np.float32)
        w2k_l = np.zeros((128, M2 * K1 * 128), np.float32)
        b2r_l = np.zeros((1, M2 * 128), np.float32)
        w2r_l = np.zeros((128, M2 * K2 * 128), np.float32)
        for M in range(M1):
            cw = c1cols[M * 128:(M + 1) * 128]
            w1k_l[:64, M * 128:(M + 1) * 128] = W1k[:, cw]
            w1k_l[64, M * 128:(M + 1) * 128] = b1[cw]
            for k in range(K1):
                rows = slice(h1_chunks[k] * 128, h1_chunks[k] * 128 + 128)
                w1r_l[:, (M * K1 + k) * 128:(M * K1 + k + 1) * 128] = W1r[rows, cw]
        for M in range(M2):
            cw = c2cols[M * 128:(M + 1) * 128]
            b2r_l[0, M * 128:(M + 1) * 128] = b2[cw]
            for k in range(K1):
                rows = slice(h1_chunks[k] * 128, h1_chunks[k] * 128 + 128)
                w2k_l[:, (M * K1 + k) * 128:(M * K1 + k + 1) * 128] = W2k[rows, cw]
            for k in range(K2):
                rows = slice(h2_chunks[k] * 128, h2_chunks[k] * 128 + 128)
                w2r_l[:, (M * K2 + k) * 128:(M * K2 + k + 1) * 128] = W2r[rows, cw]
        maps.append({
            "w1r": bf(w1r_l), "w1k": bf(w1k_l), "w2k": bf(w2k_l),
            "b2r": bf(b2r_l), "w2r": bf(w2r_l), "stateT": bf(stateT),
        })
    return maps


_CACHE = {}


def kernel(state, W1k, W1r, b1, W2k, W2r, b2, Wfc, bfc, Wv, bv):
    state = np.asarray(state, np.float32)
    T = state.shape[1]
    maps = _shard_inputs(T, state,
                         np.asarray(W1k, np.float32), np.asarray(W1r, np.float32),
                         np.asarray(b1, np.float32), np.asarray(W2k, np.float32),
                         np.asarray(W2r, np.float32), np.asarray(b2, np.float32))
    if T not in _CACHE:
        _CACHE[T] = _build_kernel(T)
    nc = _CACHE[T]
    res = run_bass_kernel_spmd(nc, maps, list(range(8)))
    h2 = np.zeros((H2, B), np.float32)
    for rho, r in ((0, res.results[0]), (1, res.results[1])):
        for c in range(2):
            h2[rho * 256 + c * 128:rho * 256 + (c + 1) * 128, :] = \
                r["out"][:, c * 32:(c + 1) * 32]
    h2 = h2.T  # [B, H2]
    value = 1.0 / (1.0 + np.exp(-(h2 @ np.asarray(Wfc, np.float32)
                                  + np.asarray(bfc, np.float32))))
    v = value @ np.asarray(Wv, np.float32) + np.asarray(bv, np.float32)
    mu = value[0:1, 0:2].copy()
    cov = value[0:1, 1:5].reshape(1, 2, 2).copy()
    return v, mu, cov
